# revision 44
# baseline (speedup 1.0000x reference)
"""Causal multi-head attention on 8 Trainium2 NeuronCores.

Problem: x[4, 2048, 1024], 16 heads of 64; q/k/v = x@W* + b*, causal
softmax attention, out = y@Wp + bp.

Sharding: core c handles batch b = c//2 and head-group hg = c%2
(8 heads = 512 feature columns of Wq/Wk/Wv, 512 rows of Wp).  Each core
computes a full [2048, 1024] partial of the output projection for its
batch; the host sums the two partials per batch and adds bp.

Per-core dataflow, all in bf16 (inputs cast on host) with fp32 PSUM
accumulation:
  * xT comes straight from DRAM via XBAR DMA transposes (no PE
    transposes, no PSUM->SBUF copies).
  * Per 512-row sequence quarter `seg`: v rows, qT/kT columns are
    projected (PE + DVE bias), then attention runs for the quarter --
    causality means quarter seg only needs k/v prefixes already
    produced, so quarter seg+1's projections and quarter seg-1's output
    projection provide PE fill while the Activation engine exponentiates.
  * Scores for a PAIR of heads land side by side in one PSUM tile
    [128, 2, 512] so a single ACT exp instruction (3D AP, which also
    skips the causally-dead left columns of diagonal blocks) serves two
    heads -- ACT is the co-bottleneck engine and pays a fixed ~185ns
    per instruction.
  * Per head: y_extT[65, q] += v_ext.T @ sT in PSUM; the appended ones
    column of v makes row 64 the softmax denominator.
  * Normalization per (pair, seg): DVE reciprocal of the denominator
    row, DRAM-roundtrip partition broadcast, and a fused
    multiply-while-copying of y from PSUM into the dead qT columns.
  * Output projection of quarter seg (yT.T @ Wp) streams to DRAM right
    after the quarter is normalized, giving the scheduler PE work that
    overlaps the next quarter's attention.
"""
import collections

import numpy as np

B, T, D = 4, 2048, 1024
NH, HD = 16, 64
NHL = 8            # heads per core
DL = NHL * HD      # 512: local qkv feature width
P = 128
QB = 512           # q block (columns of sT tiles)
NQ = T // QB       # 4
NKT = T // P       # 16 k blocks
KC = D // P        # 8 contraction chunks over model dim
FC = DL // P       # 4 chunks over local feature dim
DB = 512           # out-projection column block
ND = D // DB       # 2
NPAIR = NHL // 2   # 4 head pairs

_CACHE = {}


def _build():
    import concourse.bass as bass
    from concourse import bacc
    import concourse.mybir as mybir
    import concourse.tile as tile

    f32 = mybir.dt.float32
    f32r = mybir.dt.float32r
    bf16 = mybir.dt.bfloat16
    Exp = mybir.ActivationFunctionType.Exp

    nc = bacc.Bacc(None)
    x_d = nc.dram_tensor("x", [T, D], bf16, kind="ExternalInput")
    wq_d = nc.dram_tensor("wq", [P, KC * DL], bf16, kind="ExternalInput")
    wk_d = nc.dram_tensor("wk", [P, KC * DL], bf16, kind="ExternalInput")
    wv_d = nc.dram_tensor("wv", [P, KC * DL], bf16, kind="ExternalInput")
    wp_d = nc.dram_tensor("wp", [P, FC * D], bf16, kind="ExternalInput")
    bq_d = nc.dram_tensor("bq", [DL], f32, kind="ExternalInput")
    bk_d = nc.dram_tensor("bk", [DL], f32, kind="ExternalInput")
    bv_d = nc.dram_tensor("bv", [DL], f32, kind="ExternalInput")
    mask2_d = nc.dram_tensor("mask2", [P, 2 * P], bf16, kind="ExternalInput")
    out_d = nc.dram_tensor("out", [T, D], bf16, kind="ExternalOutput")

    def bcast_ap(ap, parts):
        """Partition-broadcast view of a DRAM AP (stride-0 partition dim)."""
        return bass.AP(tensor=ap.tensor, offset=ap.offset,
                       ap=[[0, parts]] + list(ap.ap))

    with tile.TileContext(nc) as tc:
        with (
            tc.tile_pool(name="const", bufs=1) as const,
            tc.tile_pool(name="xT", bufs=1) as xT_pool,
            tc.tile_pool(name="qkT", bufs=1) as qkT_pool,
            tc.tile_pool(name="vext", bufs=1) as v_pool,
            tc.tile_pool(name="w", bufs=1) as w_pool,
            tc.tile_pool(name="dram", bufs=1, space="DRAM") as dram_pool,
        ):
            xT_sb = xT_pool.tile([P, KC, T], bf16)
            qT_sb = qkT_pool.tile([P, FC, T], bf16)   # becomes yT in place
            kT_sb = qkT_pool.tile([P, FC, T], bf16)
            v_sb = v_pool.tile([P, NKT, NHL, HD + 1], bf16)
            ones_sb = const.tile([P, NKT, NHL], f32)
            nc.vector.memset(ones_sb, 1.0)
            nc.vector.tensor_copy(v_sb[:, :, :, HD], ones_sb)

            wv_sb = w_pool.tile([P, KC, DL], bf16)
            wq_sb = w_pool.tile([P, KC, DL], bf16)
            wk_sb = w_pool.tile([P, KC, DL], bf16)
            wp_sb = w_pool.tile([P, FC, D], bf16)
            rec_dram = dram_pool.tile([NPAIR, NQ, 2, QB], f32)

            # x transposes: seg 0's eight feature chunks first (they gate
            # the first v/qk projections), then weights, then the rest.
            def tr_seg(s):
                for c in range(KC):
                    nc.sync.dma_start_transpose(
                        xT_sb[:, c, s * QB:(s + 1) * QB],
                        x_d.ap()[s * QB:(s + 1) * QB, c * P:(c + 1) * P])

            tr_seg(0)
            nc.sync.dma_start(wv_sb,
                              wv_d.ap().rearrange("p (c m) -> p c m", c=KC))
            nc.sync.dma_start(wq_sb,
                              wq_d.ap().rearrange("p (c m) -> p c m", c=KC))
            nc.sync.dma_start(wk_sb,
                              wk_d.ap().rearrange("p (c m) -> p c m", c=KC))
            mask2_sb = const.tile([P, 2, P], bf16)
            nc.sync.dma_start(mask2_sb, mask2_d.ap())
            bq_sb = const.tile([P, FC], f32)
            nc.sync.dma_start(bq_sb, bq_d.ap().rearrange("(c p) -> p c", p=P))
            bk_sb = const.tile([P, FC], f32)
            nc.sync.dma_start(bk_sb, bk_d.ap().rearrange("(c p) -> p c", p=P))
            bv_sb = const.tile([P, DL], f32)
            nc.gpsimd.dma_start(out=bv_sb, in_=bcast_ap(bv_d.ap(), P))
            for s in range(1, NQ):
                tr_seg(s)
            nc.sync.dma_start(wp_sb,
                              wp_d.ap().rearrange("p (c m) -> p c m", c=FC))

            with (
                tc.tile_pool(name="sT", bufs=3) as sT_pool,
                tc.tile_pool(name="sums", bufs=2) as sums_pool,
                tc.tile_pool(name="rbc", bufs=2) as rbc_pool,
                tc.tile_pool(name="outsb", bufs=4) as out_pool,
                tc.tile_pool(name="ps_s", bufs=2, space="PSUM") as ps_s,
                tc.tile_pool(name="ps_y", bufs=2, space="PSUM") as ps_y,
                tc.tile_pool(name="ps_mx", bufs=2, space="PSUM") as ps_mx,
            ):

                # --- units: small issue-order chunks of PE-dominated work
                # used as filler between attention blocks (the PE queue is
                # in-order, so filler must be ISSUED ahead of instructions
                # that will stall, to absorb the ACT exp latency).

                def v_unit(jt):
                    ps = ps_mx.tile([P, DL], f32, tag="mx")
                    for kc in range(KC):
                        nc.tensor.matmul(
                            ps,
                            lhsT=xT_sb[:, kc, jt * P:(jt + 1) * P],
                            rhs=wv_sb[:, kc, :],
                            start=(kc == 0), stop=(kc == KC - 1))
                    nc.vector.tensor_tensor(
                        v_sb[:, jt, :, 0:HD],
                        ps.rearrange("p (h e) -> p h e", h=NHL),
                        bv_sb.rearrange("p (h e) -> p h e", h=NHL),
                        mybir.AluOpType.add)

                def qk_unit(seg, m, which):
                    w_sb, b_sb, dst = ((wq_sb, bq_sb, qT_sb) if which == "q"
                                       else (wk_sb, bk_sb, kT_sb))
                    ps = ps_mx.tile([P, QB], f32, tag="mx")
                    for kc in range(KC):
                        nc.tensor.matmul(
                            ps,
                            lhsT=w_sb[:, kc, m * P:(m + 1) * P],
                            rhs=xT_sb[:, kc, seg * QB:(seg + 1) * QB],
                            start=(kc == 0), stop=(kc == KC - 1))
                    nc.vector.tensor_scalar_add(
                        dst[:, m, seg * QB:(seg + 1) * QB], ps,
                        b_sb[:, m:m + 1])

                def oproj_unit(jt, nd, act_copy=False):
                    ps = ps_mx.tile([P, DB], f32, tag="mx")
                    for c in range(FC):
                        nc.tensor.matmul(
                            ps,
                            lhsT=qT_sb[:, c, jt * P:(jt + 1) * P],
                            rhs=wp_sb[:, c, nd * DB:(nd + 1) * DB],
                            start=(c == 0), stop=(c == FC - 1))
                    ot = out_pool.tile([P, DB], bf16)
                    if act_copy:
                        nc.scalar.activation(out=ot, in_=ps,
                                             func=mybir.ActivationFunctionType
                                             .Copy)
                    else:
                        nc.vector.tensor_copy(ot, ps)
                    nc.sync.dma_start(
                        out_d.ap()[jt * P:(jt + 1) * P,
                                   nd * DB:(nd + 1) * DB],
                        ot)

                fillers = collections.deque()

                def pump(n=1):
                    for _ in range(n):
                        if fillers:
                            fillers.popleft()()

                # how many attention blocks between filler pumps, per seg
                PUMP_IVL = {0: 1, 1: 3, 2: 2, 3: 5}

                def attention_pair(p, seg):
                    q0 = seg * QB
                    n_ik = 4 * seg + 4
                    psy = ps_y.tile([P, 2, QB], f32)
                    for ik in range(n_ik):
                        pd = ik - 4 * seg
                        c0 = max(0, pd * P)
                        sT = sT_pool.tile([P, 2, QB], bf16)
                        for hi in range(2):
                            psS = ps_s.tile([P, QB], f32)
                            nc.tensor.matmul(
                                psS[:, c0:QB],
                                lhsT=kT_sb[hi * HD:(hi + 1) * HD, p,
                                           ik * P:(ik + 1) * P],
                                rhs=qT_sb[hi * HD:(hi + 1) * HD, p,
                                          q0 + c0:q0 + QB],
                                start=True, stop=True)
                            nc.scalar.activation(
                                out=sT[:, hi, c0:QB], in_=psS[:, c0:QB],
                                func=Exp, scale=0.125)
                            if pd >= 0:
                                nc.vector.tensor_mul(
                                    sT[:, hi, c0:c0 + P],
                                    sT[:, hi, c0:c0 + P],
                                    mask2_sb[:, hi, :])
                        if (seg > 0 or p >= 2) and ik % PUMP_IVL[seg] == 0:
                            pump()
                        for hi in range(2):
                            nc.tensor.matmul(
                                psy[0:HD + 1, hi, c0:QB],
                                lhsT=v_sb[:, ik, 2 * p + hi, :],
                                rhs=sT[:, hi, c0:QB],
                                start=(ik == 0), stop=(ik == n_ik - 1))
                    # softmax denominators -> DVE reciprocals (f32r, so the
                    # PE broadcast matmul below may consume them) -> a
                    # ones-column matmul replicates each head's 1/denom row
                    # across 64 PSUM partitions -> y written normalized in
                    # place over the pair's dead qT columns (DVE reads psy
                    # at partitions 0:64 and writes head 1 shifted to
                    # 64:128).
                    sums = sums_pool.tile([P, QB], f32)
                    r_sb = rbc_pool.tile([P, QB], f32)
                    for hi in range(2):
                        nc.vector.reciprocal(
                            sums[32 * hi:32 * hi + 1, :],
                            psy[HD:HD + 1, hi, :])
                        nc.sync.dma_start(
                            rec_dram[p, seg, hi],
                            sums[32 * hi:32 * hi + 1, :])
                        nc.sync.dma_start(
                            out=r_sb[hi * HD:(hi + 1) * HD, :],
                            in_=bcast_ap(rec_dram[p, seg, hi], HD))
                    for hi in range(2):
                        nc.vector.tensor_mul(
                            qT_sb[hi * HD:(hi + 1) * HD, p, q0:q0 + QB],
                            psy[0:HD, hi, :],
                            r_sb[hi * HD:(hi + 1) * HD, :])

                # cold start: seg 0's projections issued directly
                for jt in range(4):
                    v_unit(jt)
                for m in range(FC):
                    qk_unit(0, m, "q")
                    qk_unit(0, m, "k")

                for seg in range(NQ):
                    if seg < NQ - 1:
                        for m in range(FC):
                            fillers.append(
                                lambda jt=4 * (seg + 1) + m: v_unit(jt))
                            fillers.append(
                                lambda s=seg + 1, m=m: qk_unit(s, m, "q"))
                            fillers.append(
                                lambda s=seg + 1, m=m: qk_unit(s, m, "k"))
                    # output projection of earlier, already-normalized
                    # quarters rides along as late-seg filler
                    for so in ([0] if seg == 2 else [1, 2] if seg == 3
                               else []):
                        for jt in range(4 * so, 4 * so + 4):
                            for nd in range(ND):
                                fillers.append(
                                    lambda jt=jt, nd=nd: oproj_unit(jt, nd))
                    for p in range(NPAIR):
                        attention_pair(p, seg)
                    if seg < NQ - 1:
                        pump(len(fillers))
                # drain: quarter 2's output projection covers the last
                # pairs' normalization latency, then the final quarter
                pump(len(fillers))
                for jt in range(12, 16):
                    for nd in range(ND):
                        oproj_unit(jt, nd, act_copy=(jt + nd) % 2 == 0)

    nc.finalize()
    return nc


def _in_maps(x, Wq, bq, Wk, bk, Wv, bv, Wp):
    import ml_dtypes
    bf16 = ml_dtypes.bfloat16

    tri = np.triu(np.ones((P, P), dtype=np.float32))  # keep q >= k
    mask2 = np.concatenate([tri, tri], axis=1).astype(bf16)

    def wqkv(w):  # [D, DL] -> [P, KC*DL] with (c p) m -> p (c m)
        return np.ascontiguousarray(
            w.reshape(KC, P, DL).transpose(1, 0, 2).reshape(P, KC * DL)
        ).astype(bf16)

    maps = []
    for c in range(8):
        b, hg = divmod(c, 2)
        sl = slice(hg * DL, (hg + 1) * DL)
        wp_l = Wp[sl, :]  # [DL, D] -> [P, FC*D]
        maps.append({
            "x": np.ascontiguousarray(x[b]).astype(bf16),
            "wq": wqkv(Wq[:, sl]),
            "wk": wqkv(Wk[:, sl]),
            "wv": wqkv(Wv[:, sl]),
            "wp": np.ascontiguousarray(
                wp_l.reshape(FC, P, D).transpose(1, 0, 2).reshape(P, FC * D)
            ).astype(bf16),
            "bq": np.ascontiguousarray(bq[sl]),
            "bk": np.ascontiguousarray(bk[sl]),
            "bv": np.ascontiguousarray(bv[sl]),
            "mask2": mask2,
        })
    return maps


def kernel(x, Wq, bq, Wk, bk, Wv, bv, Wp, bp):
    from concourse.bass_utils import run_bass_kernel_spmd

    if "nc" not in _CACHE:
        _CACHE["nc"] = _build()
    nc = _CACHE["nc"]

    x = np.asarray(x, np.float32)
    Wq, bq, Wk, bk, Wv, bv, Wp = [
        np.asarray(a, np.float32) for a in (Wq, bq, Wk, bk, Wv, bv, Wp)]
    bp = np.asarray(bp, np.float32)

    in_maps = _in_maps(x, Wq, bq, Wk, bk, Wv, bv, Wp)
    _CACHE["in_maps"] = in_maps

    res = run_bass_kernel_spmd(nc, in_maps, list(range(8))).results
    out = np.empty((B, T, D), dtype=np.float32)
    for b in range(B):
        out[b] = (res[2 * b]["out"].astype(np.float32)
                  + res[2 * b + 1]["out"].astype(np.float32) + bp)
    return out
